# revision 55
# baseline (speedup 1.0000x reference)
"""Trainium2 Bass kernel for the ragged-sequence attention problem.

Math (per batch row):
    u      = tanh(h @ W.T + b)          h: [L, H]
    s      = u @ u_s                    masked to length, then softmax
    v      = sum_l alpha_l * h_l

Strategy (v4 — ragged, transpose-free, broadcast-free, bias/denom-free):
  - Length-aware schedule: rows are LPT-packed onto 8 cores (32 rows each,
    balanced by tile count T_b = ceil(len/128)), each core's rows sorted by
    T descending.  The SPMD program processes slot i with
    T_seq[i] = max over cores of that rank's T, so one program serves all
    cores; padding overhead is ~2% of tiles.  ~293 tiles/core vs 512 dense.
  - h is pre-cast to bf16 and padded to 256 channels on the host; channel
    255 is set to ONES.  The DMA XBAR-transpose loads h c-major per
    512-l group ([128, 1024] tiles, chunk1 at column 512) straight from
    DRAM — no PE transposes, no PSUM evacuation, no on-device cast.
  - u-matmul: 4 matmuls per group (2 k-chunks x 2 m-chunks); the W.T
    pad row 255 carries the BIAS, which the ones channel multiplies in —
    so one bias-free tanh ACT covers both m-chunks of a 2-bank PSUM tile.
  - scores: the u_s column is replicated to 128 stationary columns on the
    host, so the scores matmul emits 128 IDENTICAL score rows into PSUM.
    The length mask is a K=1 matmul adding a -1e30 log-mask row (skipped
    for groups provably full on every core).  One Exp ACT per group then
    reads PSUM [128, N] and yields the alpha BROADCAST in bf16 directly.
  - v: one GpSimd tensor_mul over both h chunks ([128, 2, N], alpha
    stride-0-broadcast on the chunk dim) + one DVE tensor_reduce per
    group; a final per-row reduce folds groups.  The ones channel makes
    chunk1 partition 127 the softmax denominator for free.  Host divides
    and unpermutes.
  - Software pipeline over (row, group) units: scores lag u by 1 unit,
    exp by 2, v by 3, so the in-order engine queues never stall on
    cross-engine round trips.
"""

import sys

import numpy as np

sys.path.insert(0, "/opt/trn_rl_repo")

import ml_dtypes  # noqa: E402

import concourse.bass as bass  # noqa: E402
import concourse.mybir as mybir  # noqa: E402
import concourse.tile as tile  # noqa: E402
from concourse.bass_utils import run_bass_kernel_spmd  # noqa: E402
import bass_rust as _br  # noqa: E402

N_CORES = 8
B, L, H = 256, 2048, 240
BPC = B // N_CORES        # batch rows per core
HP = 256                  # h channels padded (two 128 chunks)
H1 = H - 128              # 112 real channels in chunk 1
NG = 4                    # max l-groups of 512
GSZ = 512
F32 = mybir.dt.float32
BF16 = mybir.dt.bfloat16
AF = mybir.ActivationFunctionType
ALU = mybir.AluOpType
AX = mybir.AxisListType
BF16NP = ml_dtypes.bfloat16

_MAXW = 1  # sync waits kept on an instruction; the rest move to nops


class _TC(tile.TileContext):
    """Walrus in this container caps sync-wait commands per instruction
    ("Too many sync wait commands"), but Tile freely attaches one wait per
    producer semaphore.  After scheduling, hoist excess waits onto dedicated
    single-wait nops inserted just before the instruction on its engine."""

    def schedule_and_allocate(self, validate_deps=False):
        ret = super().schedule_and_allocate(validate_deps)
        self._split_excess_waits()
        return ret

    def _split_excess_waits(self):
        nc = self.nc
        n_split = 0
        for fn in nc.m.functions:
            for bb in fn.blocks:
                insts = bb.instructions
                i = 0
                while i < len(insts):
                    inst = insts[i]
                    si = getattr(inst, "sync_info", None)
                    waits = list(si.on_wait) if si is not None else []
                    if len(waits) > _MAXW:
                        si.on_wait = waits[-_MAXW:]
                        inst.sync_info = si
                        for w in waits[:-_MAXW]:
                            nop = mybir.InstNoOp(
                                name=f"waitsplit-{n_split}", ins=[], outs=[])
                            n_split += 1
                            nop.engine = inst.engine
                            nop.sync_info = _br.SyncInfo(
                                on_wait=[w], on_update=[])
                            nc.register_instruction(nop, overwrite=True)
                            insts.insert(i, nop)
                            i += 1
                    i += 1


def _schedule(lens):
    """LPT-pack rows onto cores; per-core descending by tile count.
    Returns (perm [8][32] row ids, T_seq [32], full [32][4] bools)."""
    lens = np.asarray(lens).astype(np.int64)
    T = np.ceil(lens / 128).astype(np.int64)
    order = np.argsort(-T, kind="stable")
    perm = [[] for _ in range(N_CORES)]
    loads = [0] * N_CORES
    for idx in order:
        cand = sorted(range(N_CORES), key=lambda c: (loads[c], len(perm[c])))
        for c in cand:
            if len(perm[c]) < BPC:
                perm[c].append(int(idx))
                loads[c] += int(T[idx])
                break
    for c in range(N_CORES):
        perm[c].sort(key=lambda r: -int(T[r]))
    T_seq = [max(int(T[perm[c][i]]) for c in range(N_CORES))
             for i in range(BPC)]
    return perm, tuple(T_seq)


def build(T_seq):
    nc = bass.Bass("TRN2", target_bir_lowering=False, debug=False,
                   num_devices=N_CORES)
    h_d = nc.declare_dram_parameter("h", [BPC, L, HP], BF16, isOutput=False)
    w0_d = nc.declare_dram_parameter("wtb0", [128, HP], BF16, isOutput=False)
    w1_d = nc.declare_dram_parameter("wtb1", [128, HP], BF16, isOutput=False)
    u0_d = nc.declare_dram_parameter("usr0", [128, BPC * 128], BF16,
                                     isOutput=False)
    u1_d = nc.declare_dram_parameter("usr1", [128, BPC * 128], BF16,
                                     isOutput=False)
    ov_d = nc.declare_dram_parameter("ov", [BPC, 128, 2], F32, isOutput=True)

    with _TC(nc) as tc:
        with (
            tc.tile_pool(name="consts", bufs=1) as cp,
            tc.tile_pool(name="ht", bufs=1) as htp,
            tc.tile_pool(name="ut", bufs=4) as utp,
            tc.tile_pool(name="ab", bufs=4) as abp,
            tc.tile_pool(name="pr", bufs=4) as prp,
            tc.tile_pool(name="sc", bufs=4) as scp,
            tc.tile_pool(name="pu", bufs=2, space="PSUM") as pup,
            tc.tile_pool(name="sg", bufs=3, space="PSUM") as sgp,
        ):
            wtb0 = cp.tile([128, HP], BF16)
            wtb1 = cp.tile([128, HP], BF16)
            usr0 = cp.tile([128, BPC * 128], BF16)
            usr1 = cp.tile([128, BPC * 128], BF16)
            nc.gpsimd.dma_start(wtb0[:], w0_d.ap()[:, :])
            nc.gpsimd.dma_start(wtb1[:], w1_d.ap()[:, :])
            nc.gpsimd.dma_start(usr0[:], u0_d.ap()[:, :])
            nc.gpsimd.dma_start(usr1[:], u1_d.ap()[:, :])

            class Row:
                pass

            def new_row(i, T):
                # exact-sized per-row tile; ALL rows resident at once
                # (~150KB/partition total) so the DMA ring never stalls
                # on a write-after-read and the PE never waits past row 0.
                # ONE transpose per row: h[l, 256] viewed as two virtual
                # 128-channel rows [(l j), 128] makes the DRAM reads fully
                # contiguous (~350 GB/s vs 261 for the strided per-chunk
                # loads); ht comes out chunk-interleaved: ht[c, 2l+j] =
                # h[l, 128j+c], consumed via stride-2 column APs.
                r = Row()
                r.i, r.T = i, T
                r.G = (T + 3) // 4
                r.ht = htp.tile([128, 2 * 128 * T], BF16, tag=f"ht{i}")
                nc.sync.dma_start(
                    r.ht[:],
                    h_d.ap()[i, 0:128 * T, :].rearrange(
                        "l (j c) -> (l j) c", c=128),
                    transpose=True)
                r.hv = r.ht[:].rearrange("p (l k) -> p k l", k=2)
                r.ut, r.sg, r.ab = {}, {}, {}
                r.vg = scp.tile([128, 2 * NG], F32, tag="vg")
                return r

            def nsz(r, g):
                return min(GSZ, 128 * r.T - g * GSZ)

            def emit_u(r, g):
                N = nsz(r, g)
                gs = slice(g * GSZ, g * GSZ + N)
                h0 = r.hv[:, 0, gs]
                h1 = r.hv[:, 1, gs]
                pu = pup.tile([128, 2 * GSZ], F32, tag="pu")
                nc.tensor.matmul(pu[:, 0:N], wtb0[:, 0:128], h0,
                                 start=True, stop=False)
                nc.tensor.matmul(pu[:, 0:N], wtb1[:, 0:128],
                                 h1, start=False, stop=True)
                nc.tensor.matmul(pu[:, GSZ:GSZ + N], wtb0[:, 128:HP],
                                 h0, start=True, stop=False)
                nc.tensor.matmul(pu[:, GSZ:GSZ + N], wtb1[:, 128:HP],
                                 h1, start=False, stop=True)
                ut = utp.tile([128, 2 * GSZ], BF16, tag="ut")
                nc.scalar.activation(
                    ut[:].rearrange("p (k l) -> p k l", k=2)[:, :, 0:N],
                    pu[:].rearrange("p (k l) -> p k l", k=2)[:, :, 0:N],
                    AF.Tanh)
                r.ut[g] = ut

            def emit_scores(r, g):
                N = nsz(r, g)
                ut = r.ut.pop(g)
                sg = sgp.tile([128, GSZ], F32, tag="sg")
                nc.tensor.matmul(sg[:, 0:N],
                                 usr0[:, 128 * r.i:128 * r.i + 128],
                                 ut[:, 0:N], start=True, stop=False)
                nc.tensor.matmul(sg[:, 0:N],
                                 usr1[:, 128 * r.i:128 * r.i + 128],
                                 ut[:, GSZ:GSZ + N],
                                 start=False, stop=True)
                r.sg[g] = sg

            def emit_exp(r, g):
                N = nsz(r, g)
                sg = r.sg.pop(g)
                ab = abp.tile([128, GSZ], BF16, tag="ab")
                nc.scalar.activation(ab[:, 0:N], sg[:, 0:N], AF.Exp)
                r.ab[g] = ab

            def emit_v(r, g):
                N = nsz(r, g)
                ab = r.ab.pop(g)
                prod = prp.tile([128, 2 * GSZ], BF16, tag="prod")
                pview = prod[:].rearrange("p (k l) -> p k l", k=2)
                nc.vector.tensor_mul(
                    pview[:, :, 0:N],
                    r.hv[:, :, g * GSZ:g * GSZ + N],
                    ab[:, 0:N].rearrange(
                        "p (o l) -> p o l", o=1).to_broadcast((128, 2, N)))
                nc.vector.tensor_reduce(
                    r.vg[:].rearrange("p (g k) -> p g k", k=2)[:, g, :],
                    pview[:, :, 0:N], AX.X, ALU.add)
                if g == r.G - 1:
                    vfin = scp.tile([128, 2], F32, tag="vfin")
                    nc.vector.tensor_reduce(
                        vfin[:],
                        r.vg[:].rearrange("p (g k) -> p k g", k=2)[:, :, 0:r.G],
                        AX.X, ALU.add)
                    nc.gpsimd.dma_start(ov_d.ap()[r.i], vfin[:])

            # ---- software-pipelined emission over (row, group) units ----
            # shortest row first: its h lands in <1us so the PE starts
            # ~12us earlier while the big rows stream in behind it
            slot_order = [BPC - 1] + list(range(BPC - 1))
            stream = []
            for i in slot_order:
                for g in range((T_seq[i] + 3) // 4):
                    stream.append((i, g))
            rows = {}
            SLAG, ELAG, VLAG = 1, 2, 3
            for i in slot_order:
                rows[i] = new_row(i, T_seq[i])
            for k in range(len(stream) + VLAG):
                if 0 <= k < len(stream):
                    i, g = stream[k]
                    emit_u(rows[i], g)
                if 0 <= k - SLAG < len(stream):
                    i, g = stream[k - SLAG]
                    emit_scores(rows[i], g)
                if 0 <= k - ELAG < len(stream):
                    i, g = stream[k - ELAG]
                    emit_exp(rows[i], g)
                if 0 <= k - VLAG < len(stream):
                    i, g = stream[k - VLAG]
                    emit_v(rows[i], g)

    return nc


_NC_CACHE = {}


def _get_nc(T_seq):
    if T_seq not in _NC_CACHE:
        _NC_CACHE[T_seq] = build(T_seq)
    return _NC_CACHE[T_seq]


def _prep_in_maps(short_perference, current_perference, W, bvec, length_input,
                  perm, T_seq):
    h = np.asarray(short_perference, dtype=np.float32)[0]      # [B, L, H]
    us = np.asarray(current_perference, dtype=np.float32)[0]   # [B, H]
    W = np.asarray(W, dtype=np.float32)
    bvec = np.asarray(bvec, dtype=np.float32)
    lens = np.asarray(length_input).astype(np.int64)

    wt = np.zeros((HP, HP), dtype=np.float32)                  # [c, o]
    wt[:H, :H] = W.T
    wt[HP - 1, :H] = bvec                                      # bias row
    wtb0 = wt[0:128].astype(BF16NP)
    wtb1 = wt[128:HP].astype(BF16NP)

    in_maps = []
    for c in range(N_CORES):
        rows = perm[c]
        hc = np.zeros((BPC, L, HP), dtype=BF16NP)
        # h rows at l >= len are ZERO (incl. the ones/bias channel): they
        # then contribute exactly 0 to scores, numerator, and denominator
        # (u=tanh(0)=0, s=0, alpha=1, alpha*h=0, alpha*ones=0) — the
        # length mask costs nothing on device.
        for i, r in enumerate(rows):
            n = int(lens[r])
            hc[i, 0:n, 0:H] = h[r, 0:n].astype(BF16NP)
            hc[i, 0:n, HP - 1] = BF16NP(1.0)
        usc = np.zeros((HP, BPC), dtype=np.float32)
        usc[0:H, :] = us[rows].T
        usr0 = np.repeat(usc[0:128].astype(BF16NP), 128, axis=1)
        usr1 = np.repeat(usc[128:HP].astype(BF16NP), 128, axis=1)
        in_maps.append({
            "h": hc,
            "wtb0": wtb0,
            "wtb1": wtb1,
            "usr0": np.ascontiguousarray(usr0),
            "usr1": np.ascontiguousarray(usr1),
        })
    return in_maps


def run(short_perference, current_perference, W, b, length_input,
        trace=False, **run_kwargs):
    lens = np.asarray(length_input).astype(np.int64)
    perm, T_seq = _schedule(lens)
    nc = _get_nc(T_seq)
    in_maps = _prep_in_maps(short_perference, current_perference, W, b,
                            lens, perm, T_seq)
    res = run_bass_kernel_spmd(nc, in_maps, list(range(N_CORES)),
                               trace=trace, **run_kwargs)
    v = np.zeros((B, H), dtype=np.float32)
    for c in range(N_CORES):
        ov = np.asarray(res.results[c]["ov"], dtype=np.float32)  # [BPC,128,2]
        for i, r in enumerate(perm[c]):
            denom = ov[i, 127, 1]
            num = np.concatenate([ov[i, :, 0], ov[i, 0:H1, 1]])
            v[r] = num / denom
    return v, res


def kernel(short_perference, current_perference, W, b, current_batch,
           length_input):
    v, _ = run(short_perference, current_perference, W, b, length_input)
    return v.astype(np.float32)
